# revision 12
# baseline (speedup 1.0000x reference)
"""HFCAM channel-attention kernel for Trainium2 (8 NeuronCores, data-parallel on batch).

Math (per batch element, after observing that the reference's spatial permutes
cancel): with X = x[b] flattened to (N=H*W, C) in natural row-major order,
    S  = X^T @ X                  (C x C channel Gram matrix)
    M  = softmax(S, axis=-1)      (row softmax)
    out = X @ (gamma * M + I)     (gamma-scaled residual folded into the weights)

Implementation per core (one batch element):
  Phase A (streaming): load X in (128, C) spatial chunks; ACT casts hi=fp16(X);
    PE accumulates S = sum hi_chunk^T hi_chunk (fp16 matmuls, fp32 PSUM) and,
    sharing the same loaded stationary weights, computes hiT = hi_chunk^T via
    matmuls against an fp16 identity (exact transpose).  ACT/DVE evacuate the
    hiT PSUM tiles to SBUF as fp16.
  Phase B: row softmax of S (DVE reduce-max + ACT exp with fused row-sum),
    build Mp = gamma*M + I in fp32, cast to fp16.
  Phase C: per chunk, Y = hiT_chunk^T @ Mp accumulated in PSUM; evacuate with
    a scale of s = (1+gamma)/fp16(1+gamma) (corrects the fp16 rounding of the
    dominant diagonal of Mp at fp32 precision, riding the evacuation op for
    free), alternating ACT/DVE, then DMA out.

gamma is known on the host at trace time, so it is baked in as immediate
constants (the kernel is re-traced per call; correct for any input values).
"""

import sys

import numpy as np

for _p in ("/opt/trn_rl_repo", "/root/.axon_site/_ro/trn_rl_repo"):
    if _p not in sys.path:
        sys.path.append(_p)

B, H, W, C = 8, 128, 128, 256
N = H * W          # 16384 spatial positions per batch element
P = 128            # partitions / spatial chunk size
NCHUNK = N // P    # 128 chunks
GROUP = 4          # chunks per DMA/cast group in phase A
LOAD_BUFS = 6
Y_BUFS = 4
OUT_BUFS = 6
NGROUP = NCHUNK // GROUP
PAIR = 4           # chunks per PSUM tile in phase C
NPAIR = NCHUNK // PAIR
CH = C // 2        # 128, half of the channel dim (PE partition limit)


def _build(gamma: float):
    from contextlib import ExitStack

    import concourse.bass as bass  # noqa: F401
    import concourse.mybir as mybir
    import concourse.tile as tile
    from concourse import bacc

    f32 = mybir.dt.float32
    f16 = mybir.dt.float16

    # fp32-precision correction for the fp16 rounding of Mp's diagonal
    s_corr = float((1.0 + gamma) / np.float32(np.float16(np.float32(1.0 + gamma))))

    nc = bacc.Bacc("TRN2", target_bir_lowering=False)
    x_d = nc.dram_tensor("x", (N, C), f32, kind="ExternalInput")
    out_d = nc.dram_tensor("out", (N, C), f32, kind="ExternalOutput")
    ident_d = nc.inline_tensor(np.eye(P, dtype=np.float16), name="ident")
    iblk = np.zeros((P, 2, C), dtype=np.float16)
    iblk[:, 0, 0:P] = np.eye(P, dtype=np.float16)
    iblk[:, 1, P:C] = np.eye(P, dtype=np.float16)
    iblk_d = nc.inline_tensor(iblk, name="iblk")

    # (n p) c -> p n c views: partition-major with chunk index in the free dims
    x_v = x_d[:].rearrange("(n p) c -> p n c", p=P)
    out_v = out_d[:].rearrange("(n p) c -> p n c", p=P)

    with ExitStack() as ctx:
        tc = ctx.enter_context(tile.TileContext(nc))
        persist = ctx.enter_context(tc.tile_pool(name="persist", bufs=1))
        loads = ctx.enter_context(tc.tile_pool(name="loads", bufs=LOAD_BUFS))
        small = ctx.enter_context(tc.tile_pool(name="small", bufs=1))

        hiT0 = persist.tile([P, N], f16)   # X^T rows c 0..127,   32 KiB/part
        hiT1 = persist.tile([P, N], f16)   # X^T rows c 128..255, 32 KiB/part
        ident = small.tile([P, P], f16)
        nc.sync.dma_start(out=ident, in_=ident_d[:])
        iblk_t = small.tile([P, 2, C], f16)
        nc.sync.dma_start(out=iblk_t, in_=iblk_d[:])
        # warm the ACT Exp func table before it lands on the critical path
        warm = small.tile([P, 1], f32, name="warm")
        nc.scalar.activation(out=warm, in_=ident[:, 0:1],
                             func=mybir.ActivationFunctionType.Exp)

        s_ctx = ExitStack()
        s_psum = s_ctx.enter_context(tc.tile_pool(name="s_psum", bufs=1, space="PSUM"))
        s_t = s_psum.tile([P, C], f32)   # S rows c 0..127
        s_b = s_psum.tile([P, C], f32)   # S rows c 128..255

        # ---------------- Phase A ----------------
        groups = [(i * GROUP, GROUP) for i in range(NGROUP - 1)]
        groups += [(NCHUNK - GROUP + i, 1) for i in range(GROUP)]
        with tc.tile_pool(name="t_psum", bufs=2, space="PSUM") as t_psum:
            for g_i, (c0, gsz) in enumerate(groups):
                x_t = loads.tile([P, GROUP, C], f32, tag="x", name="x_t")
                x_t = x_t[:, :gsz, :]
                nc.sync.dma_start(out=x_t, in_=x_v[:, c0:c0 + gsz, :])
                hi_t = loads.tile([P, GROUP * C], f16, tag="hi", name="hi_t")
                hi_t = hi_t[:, :gsz * C]
                nc.vector.tensor_copy(out=hi_t, in_=x_t.rearrange("p k c -> p (k c)"))
                tp0 = t_psum.tile([P, GROUP * P], f32, tag="tp0", name="tp0")
                tp0 = tp0[:, :gsz * P]
                tp1 = t_psum.tile([P, GROUP * P], f32, tag="tp1", name="tp1")
                tp1 = tp1[:, :gsz * P]
                for k in range(gsz):
                    n_ch = c0 + k
                    rhs = hi_t[:, k * C:(k + 1) * C]
                    lhsT0 = hi_t[:, k * C:k * C + CH]
                    lhsT1 = hi_t[:, k * C + CH:(k + 1) * C]
                    first, last = n_ch == 0, n_ch == NCHUNK - 1
                    nc.tensor.matmul(s_t, lhsT=lhsT0, rhs=rhs, start=first, stop=last)
                    nc.tensor.matmul(tp0[:, k * P:(k + 1) * P], lhsT=lhsT0, rhs=ident,
                                     start=True, stop=True)
                    nc.tensor.matmul(s_b, lhsT=lhsT1, rhs=rhs, start=first, stop=last)
                    nc.tensor.matmul(tp1[:, k * P:(k + 1) * P], lhsT=lhsT1, rhs=ident,
                                     start=True, stop=True)
                sl = slice(c0 * P, (c0 + gsz) * P)
                nc.scalar.copy(out=hiT0[:, sl], in_=tp0)
                if g_i % 2 == 0:
                    nc.vector.tensor_copy(out=hiT1[:, sl], in_=tp1)
                else:
                    nc.scalar.copy(out=hiT1[:, sl], in_=tp1)

        # ---------------- Phase B: softmax + Mp = gamma*M + I (fp16) ------------
        mp = [small.tile([P, C], f16, name=f"mp{i}") for i in range(2)]
        for half, s_ps in enumerate((s_t, s_b)):
            negmax = small.tile([P, 1], f32, tag=f"negmax{half}")
            nc.vector.tensor_reduce(out=negmax, in_=s_ps, axis=mybir.AxisListType.X,
                                    op=mybir.AluOpType.max, negate=True)
            e_t = small.tile([P, C], f32, tag=f"e{half}")
            rowsum = small.tile([P, 1], f32, tag=f"rs{half}")
            nc.scalar.activation(out=e_t, in_=s_ps,
                                 func=mybir.ActivationFunctionType.Exp,
                                 bias=negmax, scale=1.0, accum_out=rowsum)
            rcp = small.tile([P, 1], f32, tag=f"rcp{half}")
            nc.vector.reciprocal(out=rcp, in_=rowsum)
            # rcp *= gamma; then mp = (e * rcp) + I_block in one fused op
            nc.vector.tensor_scalar_mul(out=rcp, in0=rcp, scalar1=float(gamma))
            nc.vector.scalar_tensor_tensor(out=mp[half], in0=e_t, scalar=rcp,
                                           in1=iblk_t[:, half, :],
                                           op0=mybir.AluOpType.mult,
                                           op1=mybir.AluOpType.add)
        s_ctx.close()

        # ---------------- Phase C ----------------
        with tc.tile_pool(name="y_psum", bufs=Y_BUFS, space="PSUM") as y_psum:
            outs = ctx.enter_context(tc.tile_pool(name="outs", bufs=OUT_BUFS))
            for j in range(NPAIR):
                y_ps = y_psum.tile([P, PAIR * C], f32, tag="y")
                for k in range(PAIR):
                    isl = slice((j * PAIR + k) * P, (j * PAIR + k + 1) * P)
                    nc.tensor.matmul(y_ps[:, k * C:(k + 1) * C],
                                     lhsT=hiT0[:, isl], rhs=mp[0],
                                     start=True, stop=False)
                    nc.tensor.matmul(y_ps[:, k * C:(k + 1) * C],
                                     lhsT=hiT1[:, isl], rhs=mp[1],
                                     start=False, stop=True)
                o_t = outs.tile([P, PAIR, C], f32, tag="o")
                o_flat = o_t.rearrange("p k c -> p (k c)")
                if j % 2 == 0:
                    nc.scalar.mul(out=o_flat, in_=y_ps, mul=s_corr)
                else:
                    nc.vector.tensor_scalar_mul(out=o_flat, in0=y_ps, scalar1=s_corr)
                nc.sync.dma_start(out=out_v[:, j * PAIR:(j + 1) * PAIR, :], in_=o_t)

    nc.compile()
    return nc


def kernel(x: np.ndarray, gamma: np.ndarray) -> np.ndarray:
    from concourse import bass_utils

    assert x.shape == (B, H, W, C), x.shape
    g = float(np.asarray(gamma))
    nc = _build(g)
    in_maps = [
        {"x": np.ascontiguousarray(x[b].reshape(N, C), dtype=np.float32)}
        for b in range(B)
    ]
    res = bass_utils.run_bass_kernel_spmd(nc, in_maps, core_ids=list(range(B)))
    out = np.stack([res.results[b]["out"].reshape(H, W, C) for b in range(B)])
    return out.astype(np.float32)


if __name__ == "__main__":
    rng = np.random.default_rng(0)
    x = rng.standard_normal((B, H, W, C), dtype=np.float32)
    gamma = np.float32(0.5)
    out = kernel(x, gamma)
    print("out", out.shape, out.dtype, float(np.abs(out).max()))


# revision 14
# speedup vs baseline: 21242.8942x; 21242.8942x over previous
"""HFCAM channel-attention kernel for Trainium2 (8 NeuronCores, data-parallel on batch).

Math (per batch element, after observing that the reference's spatial permutes
cancel): with X = x[b] flattened to (N=H*W, C) in natural row-major order,
    S  = X^T @ X                  (C x C channel Gram matrix)
    M  = softmax(S, axis=-1)      (row softmax)
    out = X @ (gamma * M + I)     (gamma-scaled residual folded into the weights)

Implementation per core (one batch element), phases pipelined by the Tile
scheduler:
  Phase A (streaming, DMA/PE co-saturated): load X in (128, C) spatial chunks
    (4-chunk 512 KiB DMAs; single-chunk groups at the tail to shorten the
    critical path into phase B); DVE casts hi=fp16(X) (2x_2P mode); PE
    accumulates S = sum hi_chunk^T hi_chunk (fp16 matmuls, fp32 PSUM) and,
    sharing the same loaded stationary weights, computes hiT = hi_chunk^T via
    matmuls against an fp16 identity (exact transpose, avoids the slow
    transpose paths).  ACT (plus DVE on alternate groups) evacuates the hiT
    PSUM tiles to SBUF as fp16.
  Phase B: row softmax of S (DVE reduce-max with negate -> ACT exp with fused
    row-sum accumulator -> DVE reciprocal), then Mp = gamma*M + I_block in one
    fused scalar_tensor_tensor writing fp16 (I_block is an inline-const).  The
    ACT Exp table is preloaded at kernel start to keep it off this path.
  Phase C (store-bound): per chunk, Y = hiT_chunk^T @ Mp accumulated in PSUM
    over the two channel halves; evacuate with a scale of
    s = (1+gamma)/fp16(1+gamma) (corrects the fp16 rounding of Mp's dominant
    diagonal at fp32 precision, riding the evacuation op for free),
    alternating ACT/DVE per pair, then DMA out.

Accuracy vs the fp32 reference: ~3.6e-4 scale-relative absmax (fp16 input
rounding floor).  Cost-model time ~109 us/core vs a ~97 us pure
load+store roofline kernel.

gamma is known on the host at trace time, so it is baked in as immediate
constants (the kernel is re-traced per call; correct for any input values).
"""

import sys

import numpy as np

for _p in ("/opt/trn_rl_repo", "/root/.axon_site/_ro/trn_rl_repo"):
    if _p not in sys.path:
        sys.path.append(_p)

B, H, W, C = 8, 128, 128, 256
N = H * W          # 16384 spatial positions per batch element
P = 128            # partitions / spatial chunk size
NCHUNK = N // P    # 128 chunks
GROUP = 4          # chunks per DMA/cast group in phase A
LOAD_BUFS = 6
Y_BUFS = 8
OUT_BUFS = 6
NGROUP = NCHUNK // GROUP
PAIR = 2           # chunks per PSUM tile in phase C
NPAIR = NCHUNK // PAIR
CH = C // 2        # 128, half of the channel dim (PE partition limit)


def _build(gamma: float):
    from contextlib import ExitStack

    import concourse.bass as bass  # noqa: F401
    import concourse.mybir as mybir
    import concourse.tile as tile
    from concourse import bacc

    f32 = mybir.dt.float32
    f16 = mybir.dt.float16

    # fp32-precision correction for the fp16 rounding of Mp's diagonal
    _d16 = np.float32(np.float16(np.float32(1.0 + gamma)))
    s_corr = float((1.0 + gamma) / _d16) if abs(float(_d16)) > 1e-6 else 1.0

    nc = bacc.Bacc("TRN2", target_bir_lowering=False)
    x_d = nc.dram_tensor("x", (N, C), f32, kind="ExternalInput")
    out_d = nc.dram_tensor("out", (N, C), f32, kind="ExternalOutput")
    ident_d = nc.inline_tensor(np.eye(P, dtype=np.float16), name="ident")
    iblk = np.zeros((P, 2, C), dtype=np.float16)
    iblk[:, 0, 0:P] = np.eye(P, dtype=np.float16)
    iblk[:, 1, P:C] = np.eye(P, dtype=np.float16)
    iblk_d = nc.inline_tensor(iblk, name="iblk")

    # (n p) c -> p n c views: partition-major with chunk index in the free dims
    x_v = x_d[:].rearrange("(n p) c -> p n c", p=P)
    out_v = out_d[:].rearrange("(n p) c -> p n c", p=P)

    with ExitStack() as ctx:
        tc = ctx.enter_context(tile.TileContext(nc))
        persist = ctx.enter_context(tc.tile_pool(name="persist", bufs=1))
        loads = ctx.enter_context(tc.tile_pool(name="loads", bufs=LOAD_BUFS))
        small = ctx.enter_context(tc.tile_pool(name="small", bufs=1))

        hiT0 = persist.tile([P, N], f16)   # X^T rows c 0..127,   32 KiB/part
        hiT1 = persist.tile([P, N], f16)   # X^T rows c 128..255, 32 KiB/part
        ident = small.tile([P, P], f16)
        nc.sync.dma_start(out=ident, in_=ident_d[:])
        iblk_t = small.tile([P, 2, C], f16)
        nc.sync.dma_start(out=iblk_t, in_=iblk_d[:])
        # warm the ACT Exp func table before it lands on the critical path
        warm = small.tile([P, 1], f32, name="warm")
        nc.scalar.activation(out=warm, in_=ident[:, 0:1],
                             func=mybir.ActivationFunctionType.Exp)

        s_ctx = ExitStack()
        s_psum = s_ctx.enter_context(tc.tile_pool(name="s_psum", bufs=1, space="PSUM"))
        s_t = s_psum.tile([P, C], f32)   # S rows c 0..127
        s_b = s_psum.tile([P, C], f32)   # S rows c 128..255

        # ---------------- Phase A ----------------
        groups = [(i * GROUP, GROUP) for i in range(NGROUP - 1)]
        groups += [(NCHUNK - GROUP + i, 1) for i in range(GROUP)]
        with tc.tile_pool(name="t_psum", bufs=2, space="PSUM") as t_psum:
            for g_i, (c0, gsz) in enumerate(groups):
                x_t = loads.tile([P, GROUP, C], f32, tag="x", name="x_t")
                x_t = x_t[:, :gsz, :]
                nc.sync.dma_start(out=x_t, in_=x_v[:, c0:c0 + gsz, :])
                hi_t = loads.tile([P, GROUP * C], f16, tag="hi", name="hi_t")
                hi_t = hi_t[:, :gsz * C]
                nc.vector.tensor_copy(out=hi_t, in_=x_t.rearrange("p k c -> p (k c)"))
                tp0 = t_psum.tile([P, GROUP * P], f32, tag="tp0", name="tp0")
                tp0 = tp0[:, :gsz * P]
                tp1 = t_psum.tile([P, GROUP * P], f32, tag="tp1", name="tp1")
                tp1 = tp1[:, :gsz * P]
                for k in range(gsz):
                    n_ch = c0 + k
                    rhs = hi_t[:, k * C:(k + 1) * C]
                    lhsT0 = hi_t[:, k * C:k * C + CH]
                    lhsT1 = hi_t[:, k * C + CH:(k + 1) * C]
                    first, last = n_ch == 0, n_ch == NCHUNK - 1
                    nc.tensor.matmul(s_t, lhsT=lhsT0, rhs=rhs, start=first, stop=last)
                    nc.tensor.matmul(tp0[:, k * P:(k + 1) * P], lhsT=lhsT0, rhs=ident,
                                     start=True, stop=True)
                    nc.tensor.matmul(s_b, lhsT=lhsT1, rhs=rhs, start=first, stop=last)
                    nc.tensor.matmul(tp1[:, k * P:(k + 1) * P], lhsT=lhsT1, rhs=ident,
                                     start=True, stop=True)
                sl = slice(c0 * P, (c0 + gsz) * P)
                nc.scalar.copy(out=hiT0[:, sl], in_=tp0)
                if g_i % 2 == 0:
                    nc.vector.tensor_copy(out=hiT1[:, sl], in_=tp1)
                else:
                    nc.scalar.copy(out=hiT1[:, sl], in_=tp1)

        # ---------------- Phase B: softmax + Mp = gamma*M + I (fp16) ------------
        mp = [small.tile([P, C], f16, name=f"mp{i}") for i in range(2)]
        for half, s_ps in enumerate((s_t, s_b)):
            negmax = small.tile([P, 1], f32, tag=f"negmax{half}")
            nc.vector.tensor_reduce(out=negmax, in_=s_ps, axis=mybir.AxisListType.X,
                                    op=mybir.AluOpType.max, negate=True)
            e_t = small.tile([P, C], f32, tag=f"e{half}")
            rowsum = small.tile([P, 1], f32, tag=f"rs{half}")
            nc.scalar.activation(out=e_t, in_=s_ps,
                                 func=mybir.ActivationFunctionType.Exp,
                                 bias=negmax, scale=1.0, accum_out=rowsum)
            rcp = small.tile([P, 1], f32, tag=f"rcp{half}")
            nc.vector.reciprocal(out=rcp, in_=rowsum)
            # rcp *= gamma; then mp = (e * rcp) + I_block in one fused op
            nc.vector.tensor_scalar_mul(out=rcp, in0=rcp, scalar1=float(gamma))
            nc.vector.scalar_tensor_tensor(out=mp[half], in0=e_t, scalar=rcp,
                                           in1=iblk_t[:, half, :],
                                           op0=mybir.AluOpType.mult,
                                           op1=mybir.AluOpType.add)
        s_ctx.close()

        # ---------------- Phase C ----------------
        with tc.tile_pool(name="y_psum", bufs=Y_BUFS, space="PSUM") as y_psum:
            outs = ctx.enter_context(tc.tile_pool(name="outs", bufs=OUT_BUFS))
            for j in range(NPAIR):
                y_ps = y_psum.tile([P, PAIR * C], f32, tag="y")
                for k in range(PAIR):
                    isl = slice((j * PAIR + k) * P, (j * PAIR + k + 1) * P)
                    nc.tensor.matmul(y_ps[:, k * C:(k + 1) * C],
                                     lhsT=hiT0[:, isl], rhs=mp[0],
                                     start=True, stop=False)
                    nc.tensor.matmul(y_ps[:, k * C:(k + 1) * C],
                                     lhsT=hiT1[:, isl], rhs=mp[1],
                                     start=False, stop=True)
                o_t = outs.tile([P, PAIR, C], f32, tag="o")
                o_flat = o_t.rearrange("p k c -> p (k c)")
                if j % 2 == 0:
                    nc.scalar.mul(out=o_flat, in_=y_ps, mul=s_corr)
                else:
                    nc.vector.tensor_scalar_mul(out=o_flat, in0=y_ps, scalar1=s_corr)
                nc.sync.dma_start(out=out_v[:, j * PAIR:(j + 1) * PAIR, :], in_=o_t)

    nc.compile()
    return nc


def kernel(x: np.ndarray, gamma: np.ndarray) -> np.ndarray:
    from concourse import bass_utils

    assert x.shape == (B, H, W, C), x.shape
    g = float(np.asarray(gamma))
    nc = _build(g)
    in_maps = [
        {"x": np.ascontiguousarray(x[b].reshape(N, C), dtype=np.float32)}
        for b in range(B)
    ]
    res = bass_utils.run_bass_kernel_spmd(nc, in_maps, core_ids=list(range(B)))
    out = np.stack([res.results[b]["out"].reshape(H, W, C) for b in range(B)])
    return out.astype(np.float32)


if __name__ == "__main__":
    rng = np.random.default_rng(0)
    x = rng.standard_normal((B, H, W, C), dtype=np.float32)
    gamma = np.float32(0.5)
    out = kernel(x, gamma)
    print("out", out.shape, out.dtype, float(np.abs(out).max()))
